# revision 6
# baseline (speedup 1.0000x reference)
"""Trainium2 Bass kernel for nn_AttackHead (GNN edge/army scorer).

Strategy (8 NeuronCores, data-parallel over the 1M edges):
  - Host: split edges 8 ways; within each core bucket edges by (src-half,
    tgt-half) of the node table so gathers can use int16 indices with a
    mid-table base offset; pad each bucket to a fixed capacity so all cores
    run one identical SPMD program.
  - Device (per core): batched SWDGE dma_gather of bf16 node rows (src and
    tgt), PE transposes to feature-major, bf16 matmul for the first MLP
    layers, fp32r matmul for the second layers (edge scorer + army scorer
    fused into one [65 x e] output), ReLU/bias on ACT/DVE, additive masking,
    PE pair-transposes back to edge-major, contiguous DMA out.
  - Host: un-permute outputs back to original edge order.
"""

import numpy as np
import ml_dtypes

import concourse.bass as bass
import concourse.mybir as mybir
import concourse.tile as tile
import concourse.bacc as bacc
from concourse.bass_utils import run_bass_kernel_spmd
from concourse.library_config import mlp
from concourse.masks import make_identity

BF16 = ml_dtypes.bfloat16

N, D, K, E = 100000, 128, 64, 1000000
NCORES = 8
EC = E // NCORES            # 125000 edges per core
SPLIT = 50000               # node-table half split
SBASE = (32768, 75000)      # gather base row per half (idx16 = node - base)
SEG_CAP = 32768             # capacity per (src-half, tgt-half) bucket
EP = 4 * SEG_CAP            # 131072 padded edges per core
SUP = 512                   # edges per supertile
NSUP = EP // SUP            # 256
BATCH = 4096                # edges per dma_gather
NBAT = EP // BATCH          # 32
BPS = SEG_CAP // BATCH      # batches per segment = 8
SPB = BATCH // SUP          # supertiles per batch = 8
IDXC = BATCH // 16          # idx columns per batch = 256

_NC_CACHE = {}
_LAST_IN_MAPS = None


def _build_nc():
    if "nc" in _NC_CACHE:
        return _NC_CACHE["nc"]
    f32 = mybir.dt.float32
    f32r = mybir.dt.float32r
    bf16 = mybir.dt.bfloat16
    i16 = mybir.dt.int16

    nc = bacc.Bacc("TRN2", debug=False, num_devices=NCORES)
    node_d = nc.dram_tensor("node", [N, D], bf16, kind="ExternalInput")
    sidx_d = nc.dram_tensor("sidx", [128, NBAT * IDXC], i16, kind="ExternalInput")
    tidx_d = nc.dram_tensor("tidx", [128, NBAT * IDXC], i16, kind="ExternalInput")
    thr_d = nc.dram_tensor("thr", [128, NSUP * 4], f32, kind="ExternalInput")
    npen_d = nc.dram_tensor("npen", [128, NSUP * 4], f32, kind="ExternalInput")
    iota_d = nc.dram_tensor("iota", [128, 256], f32, kind="ExternalInput")
    w1a_d = nc.dram_tensor("w1a", [128, 128], bf16, kind="ExternalInput")
    w1b_d = nc.dram_tensor("w1b", [128, 128], bf16, kind="ExternalInput")
    a1a_d = nc.dram_tensor("a1a", [128, 128], bf16, kind="ExternalInput")
    a1b_d = nc.dram_tensor("a1b", [128, 128], bf16, kind="ExternalInput")
    m2a_d = nc.dram_tensor("m2a", [128, 65], f32, kind="ExternalInput")
    m2b_d = nc.dram_tensor("m2b", [128, 65], f32, kind="ExternalInput")
    b1_d = nc.dram_tensor("b1c", [128, 1], f32, kind="ExternalInput")
    ab1_d = nc.dram_tensor("ab1c", [128, 1], f32, kind="ExternalInput")
    b2c_d = nc.dram_tensor("b2c", [65, 1], f32, kind="ExternalInput")
    out_d = nc.dram_tensor("out65", [EP, 65], f32, kind="ExternalOutput")

    with tile.TileContext(nc) as tc:
        with (
            tc.tile_pool(name="const", bufs=1) as cp,
            tc.tile_pool(name="gat", bufs=2) as gp,
            tc.tile_pool(name="work", bufs=2) as wp,
            tc.tile_pool(name="ps", bufs=1, space="PSUM") as pp,
            tc.tile_pool(name="ps2", bufs=2, space="PSUM") as pp2,
        ):
            nc.gpsimd.load_library(mlp)
            # ---- constants
            sidx_t = cp.tile([128, NBAT * IDXC], i16)
            nc.sync.dma_start(sidx_t[:], sidx_d[:])
            tidx_t = cp.tile([128, NBAT * IDXC], i16)
            nc.sync.dma_start(tidx_t[:], tidx_d[:])
            thr_t = cp.tile([128, NSUP * 4], f32)
            nc.sync.dma_start(thr_t[:], thr_d[:])
            npen_t = cp.tile([128, NSUP * 4], f32)
            nc.sync.dma_start(npen_t[:], npen_d[:])
            iota_t = cp.tile([128, 256], f32)
            nc.sync.dma_start(iota_t[:], iota_d[:])
            w1a_t = cp.tile([128, 128], bf16)
            nc.sync.dma_start(w1a_t[:], w1a_d[:])
            w1b_t = cp.tile([128, 128], bf16)
            nc.sync.dma_start(w1b_t[:], w1b_d[:])
            a1a_t = cp.tile([128, 128], bf16)
            nc.sync.dma_start(a1a_t[:], a1a_d[:])
            a1b_t = cp.tile([128, 128], bf16)
            nc.sync.dma_start(a1b_t[:], a1b_d[:])
            m2a_f = cp.tile([128, 65], f32)
            nc.sync.dma_start(m2a_f[:], m2a_d[:])
            m2b_f = cp.tile([128, 65], f32)
            nc.sync.dma_start(m2b_f[:], m2b_d[:])
            m2a_t = cp.tile([128, 65], f32r)
            nc.vector.tensor_copy(m2a_t[:], m2a_f[:])
            m2b_t = cp.tile([128, 65], f32r)
            nc.vector.tensor_copy(m2b_t[:], m2b_f[:])
            b1_t = cp.tile([128, 1], f32)
            nc.sync.dma_start(b1_t[:], b1_d[:])
            ab1_t = cp.tile([128, 1], f32)
            nc.sync.dma_start(ab1_t[:], ab1_d[:])
            b2c_t = cp.tile([65, 1], f32)
            nc.sync.dma_start(b2c_t[:], b2c_d[:])
            ident_t = cp.tile([128, 128], bf16)
            make_identity(nc, ident_t[:])
            id65_t = cp.tile([65, 65], f32)
            make_identity(nc, id65_t[:])

            for b in range(NBAT):
                seg = b // BPS
                sh, th = seg >> 1, seg & 1
                gsrc = gp.tile([128, BATCH // 128, 128], bf16, tag="gsrc")
                nc.gpsimd.dma_gather(
                    gsrc[:], node_d[SBASE[sh]:, :],
                    sidx_t[:, b * IDXC:(b + 1) * IDXC], BATCH, BATCH, 128,
                    single_packet=False,
                )
                gtgt = gp.tile([128, BATCH // 128, 128], bf16, tag="gtgt")
                nc.gpsimd.dma_gather(
                    gtgt[:], node_d[SBASE[th]:, :],
                    tidx_t[:, b * IDXC:(b + 1) * IDXC], BATCH, BATCH, 128,
                    single_packet=False,
                )
                for s in range(SPB):
                    st = b * SPB + s
                    # ---- feature-major transposes of gathered rows
                    sT_ps = pp.tile([128, 512], bf16, space="PSUM", tag="sT")
                    tT_ps = pp.tile([128, 512], bf16, space="PSUM", tag="tT")
                    for j in range(4):
                        nc.tensor.transpose(
                            sT_ps[:, 128 * j:128 * (j + 1)],
                            gsrc[:, 4 * s + j, :], ident_t[:])
                    for j in range(4):
                        nc.tensor.transpose(
                            tT_ps[:, 128 * j:128 * (j + 1)],
                            gtgt[:, 4 * s + j, :], ident_t[:])
                    sT = wp.tile([128, 512], bf16, tag="sT_sb")
                    nc.scalar.activation(
                        sT[:], sT_ps[:], mybir.ActivationFunctionType.Copy)
                    tT = wp.tile([128, 512], bf16, tag="tT_sb")
                    nc.scalar.activation(
                        tT[:], tT_ps[:], mybir.ActivationFunctionType.Copy)
                    # ---- layer 1 (both MLPs), hidden-major
                    h1_ps = pp.tile([128, 512], f32, space="PSUM", tag="h1")
                    nc.tensor.matmul(h1_ps[:], w1a_t[:], sT[:], start=True, stop=False)
                    nc.tensor.matmul(h1_ps[:], w1b_t[:], tT[:], start=False, stop=True)
                    h2_ps = pp.tile([128, 512], f32, space="PSUM", tag="h2")
                    nc.tensor.matmul(h2_ps[:], a1a_t[:], sT[:], start=True, stop=False)
                    nc.tensor.matmul(h2_ps[:], a1b_t[:], tT[:], start=False, stop=True)
                    h1 = wp.tile([128, 512], f32r, tag="h1_sb")
                    nc.scalar.activation(
                        h1[:], h1_ps[:], mybir.ActivationFunctionType.Relu,
                        bias=b1_t[:, 0:1])
                    h2 = wp.tile([128, 512], f32r, tag="h2_sb")
                    nc.vector.tensor_scalar(
                        h2[:], h2_ps[:], ab1_t[:, 0:1], 0.0,
                        mybir.AluOpType.add, mybir.AluOpType.max)
                    # ---- layer 2 fused: rows 0..63 army, row 64 edge logit
                    o_ps = pp.tile([65, 512], f32, space="PSUM", tag="o65")
                    nc.tensor.matmul(o_ps[:], m2a_t[:], h1[:], start=True, stop=False)
                    nc.tensor.matmul(o_ps[:], m2b_t[:], h2[:], start=False, stop=True)
                    o_sb = wp.tile([65, 512], f32, tag="o_sb")
                    nc.scalar.activation(
                        o_sb[:], o_ps[:], mybir.ActivationFunctionType.Identity,
                        bias=b2c_t[:, 0:1])
                    # ---- back to edge-major, paired so DMA runs are 520B
                    fin_ps = pp2.tile([128, 260], f32, space="PSUM", tag="fin")
                    for g in range(2):
                        for c in range(2):
                            q = 2 * g + c
                            nc.tensor.transpose(
                                fin_ps[:, 65 * q:65 * q + 65],
                                o_sb[:, 256 * g + c:256 * (g + 1):2],
                                id65_t[:])
                    # ---- additive mask: addm[:, 65q+j] = -1e9*(j > thr_q), el col = npen
                    addm = wp.tile([128, 260], f32, tag="addm")
                    for q in range(4):
                        nc.vector.tensor_scalar(
                            addm[:, 65 * q:65 * q + 64],
                            iota_t[:, 0:64],
                            thr_t[:, 4 * st + q:4 * st + q + 1], -1.0e9,
                            mybir.AluOpType.is_gt, mybir.AluOpType.mult)
                    nc.vector.tensor_copy(
                        addm[:].rearrange("p (q x) -> p q x", q=4)[:, :, 64:65],
                        npen_t[:, 4 * st:4 * st + 4].broadcast_to([128, 4, 1]))
                    fin = wp.tile([128, 260], f32, tag="fin_sb")
                    nc.vector.tensor_tensor(
                        fin[:], fin_ps[:], addm[:], mybir.AluOpType.add)
                    # ---- store (rows: 256*g + 2*p + c of this supertile)
                    o_ap = out_d[st * SUP:(st + 1) * SUP, :].rearrange(
                        "(g p c) k -> p g (c k)", g=2, p=128, c=2)
                    nc.sync.dma_start(
                        o_ap, fin[:].rearrange("p (g x) -> p g x", g=2))

    nc.compile()
    _NC_CACHE["nc"] = nc
    return nc


def _prep_core(edges, army_counts):
    """Bucket/pad one core's edges; build all per-core device inputs."""
    src = edges[:, 0].astype(np.int64)
    tgt = edges[:, 1].astype(np.int64)
    sh = (src >= SPLIT).astype(np.int64)
    th = (tgt >= SPLIT).astype(np.int64)
    seg = 2 * sh + th
    order = np.argsort(seg, kind="stable")  # bucketed position -> original idx
    counts = np.bincount(seg, minlength=4)
    if counts.max() > SEG_CAP:
        raise RuntimeError(f"bucket overflow: {counts}")

    src_p = np.empty(EP, np.int64)
    tgt_p = np.empty(EP, np.int64)
    base_s = np.empty(EP, np.int64)
    base_t = np.empty(EP, np.int64)
    thr_p = np.zeros(EP, np.float32)
    npen_p = np.zeros(EP, np.float32)
    slot_of_edge = np.empty(EC, np.int64)

    sa = army_counts[src].astype(np.float32)
    ta = army_counts[tgt].astype(np.float32)
    bad = ((sa <= 2) | (ta >= 3 * sa)).astype(np.float32)
    selfe = (src == tgt).astype(np.float32)
    thr_e = sa - 1.0
    npen_e = -(bad + 100.0 * selfe)

    ofs = 0
    for q in range(4):
        qs, qt = q >> 1, q & 1
        lo = q * SEG_CAP
        n = counts[q]
        idxq = order[ofs:ofs + n]
        ofs += n
        sl = slice(lo, lo + n)
        src_p[sl] = src[idxq]
        tgt_p[sl] = tgt[idxq]
        thr_p[sl] = thr_e[idxq]
        npen_p[sl] = npen_e[idxq]
        slot_of_edge[idxq] = lo + np.arange(n)
        pad = slice(lo + n, lo + SEG_CAP)
        src_p[pad] = SBASE[qs]
        tgt_p[pad] = SBASE[qt]
        base_s[lo:lo + SEG_CAP] = SBASE[qs]
        base_t[lo:lo + SEG_CAP] = SBASE[qt]

    si = src_p - base_s  # int16-range signed offsets
    ti = tgt_p - base_t
    # ensure the LAST slot of every gather batch has non-negative src AND tgt
    # offsets (trailing-negative indices truncate the gather on device)
    inv_slot = None
    for b in range(NBAT):
        last = (b + 1) * BATCH - 1
        if si[last] >= 0 and ti[last] >= 0:
            continue
        blk = slice(b * BATCH, (b + 1) * BATCH)
        cand = np.nonzero((si[blk] >= 0) & (ti[blk] >= 0))[0]
        if cand.size == 0:
            raise RuntimeError("no swap candidate in batch")
        j = b * BATCH + cand[0]
        for arr in (src_p, tgt_p, si, ti, thr_p, npen_p):
            arr[j], arr[last] = arr[last], arr[j]
        if inv_slot is None:
            inv_slot = np.full(EP, -1, np.int64)
            inv_slot[slot_of_edge] = np.arange(EC)
        ej, el = inv_slot[j], inv_slot[last]
        if ej >= 0:
            slot_of_edge[ej] = last
        if el >= 0:
            slot_of_edge[el] = j
        inv_slot[j], inv_slot[last] = el, ej

    def wrap_idx(a):
        cols = a.astype(np.int16).reshape(-1, 16).T  # [16, EP/16]
        return np.ascontiguousarray(np.tile(cols, (8, 1)))  # [128, EP/16]

    # [p, 4*st + 2*g + c] <- edge 512*st + 256*g + 2*p + c
    def edge_tile(a):
        t = a.reshape(NSUP, 2, 128, 2)
        return np.ascontiguousarray(t.transpose(2, 0, 1, 3).reshape(128, NSUP * 4))

    return {
        "sidx": wrap_idx(si),
        "tidx": wrap_idx(ti),
        "thr": edge_tile(thr_p),
        "npen": edge_tile(npen_p),
    }, slot_of_edge


def kernel(node_embeddings, action_edges, army_counts,
           W1, b1, W2, b2, A1, ab1, A2, ab2):
    node_bf = np.asarray(node_embeddings, np.float32).astype(BF16)
    W1 = np.asarray(W1, np.float32)
    A1 = np.asarray(A1, np.float32)
    m2a = np.zeros((128, 65), np.float32)
    m2a[:, 64] = np.asarray(W2, np.float32)[:, 0]
    m2b = np.zeros((128, 65), np.float32)
    m2b[:, :64] = np.asarray(A2, np.float32)
    b2c = np.concatenate(
        [np.asarray(ab2, np.float32), np.asarray(b2, np.float32)])[:, None]
    iota = np.tile(np.arange(64, dtype=np.float32), 4)[None, :].repeat(128, 0)
    shared = {
        "node": node_bf,
        "iota": np.ascontiguousarray(iota),
        "w1a": W1[:128].astype(BF16), "w1b": W1[128:].astype(BF16),
        "a1a": A1[:128].astype(BF16), "a1b": A1[128:].astype(BF16),
        "m2a": m2a, "m2b": m2b,
        "b1c": np.asarray(b1, np.float32)[:, None],
        "ab1c": np.asarray(ab1, np.float32)[:, None],
        "b2c": b2c,
    }

    in_maps = []
    slots = []
    ac = np.asarray(army_counts)
    edges = np.asarray(action_edges)
    for c in range(NCORES):
        per, slot_of_edge = _prep_core(edges[c * EC:(c + 1) * EC], ac)
        in_maps.append({**shared, **per})
        slots.append(slot_of_edge)

    global _LAST_IN_MAPS
    _LAST_IN_MAPS = in_maps
    nc = _build_nc()
    res = run_bass_kernel_spmd(nc, in_maps, core_ids=list(range(NCORES)))

    edge_logits = np.empty(E, np.float32)
    army_logits = np.empty((E, K), np.float32)
    for c in range(NCORES):
        out65 = res.results[c]["out65"]
        sl = slots[c]
        edge_logits[c * EC:(c + 1) * EC] = out65[sl, 64]
        army_logits[c * EC:(c + 1) * EC] = out65[sl, :64]
    return edge_logits, army_logits


# revision 7
# speedup vs baseline: 1.0302x; 1.0302x over previous
"""Trainium2 Bass kernel for nn_AttackHead (GNN edge/army scorer).

Strategy (8 NeuronCores, data-parallel over the 1M edges):
  - Host: split edges 8 ways; within each core bucket edges by (src-half,
    tgt-half) of the node table so gathers can use int16 indices with a
    mid-table base offset; pad each bucket to a fixed capacity so all cores
    run one identical SPMD program.
  - Device (per core): batched SWDGE dma_gather of bf16 node rows (src and
    tgt), PE transposes to feature-major, bf16 matmul for the first MLP
    layers, fp32r matmul for the second layers (edge scorer + army scorer
    fused into one [65 x e] output), ReLU/bias on ACT/DVE, additive masking,
    PE pair-transposes back to edge-major, contiguous DMA out.
  - Host: un-permute outputs back to original edge order.
"""

import os
import numpy as np
import ml_dtypes

import concourse.bass as bass
import concourse.mybir as mybir
import concourse.tile as tile
import concourse.bacc as bacc
from concourse.bass_utils import run_bass_kernel_spmd
from concourse.library_config import mlp
from concourse.masks import make_identity

BF16 = ml_dtypes.bfloat16

N, D, K, E = 100000, 128, 64, 1000000
NCORES = 8
EC = E // NCORES            # 125000 edges per core
SPLIT = 50000               # node-table half split
SBASE = (32768, 75000)      # gather base row per half (idx16 = node - base)
SEG_CAP = 32768             # capacity per (src-half, tgt-half) bucket
EP = 4 * SEG_CAP            # 131072 padded edges per core
SUP = 512                   # edges per supertile
NSUP = EP // SUP            # 256
BATCH = 4096                # edges per dma_gather
NBAT = EP // BATCH          # 32
BPS = SEG_CAP // BATCH      # batches per segment = 8
SPB = BATCH // SUP          # supertiles per batch = 8
IDXC = BATCH // 16          # idx columns per batch = 256

_NC_CACHE = {}
_LAST_IN_MAPS = None


def _build_nc():
    if "nc" in _NC_CACHE:
        return _NC_CACHE["nc"]
    f32 = mybir.dt.float32
    f32r = mybir.dt.float32r
    bf16 = mybir.dt.bfloat16
    i16 = mybir.dt.int16

    nc = bacc.Bacc("TRN2", debug=False, num_devices=NCORES)
    node_d = nc.dram_tensor("node", [N, D], bf16, kind="ExternalInput")
    sidx_d = nc.dram_tensor("sidx", [128, NBAT * IDXC], i16, kind="ExternalInput")
    tidx_d = nc.dram_tensor("tidx", [128, NBAT * IDXC], i16, kind="ExternalInput")
    thr_d = nc.dram_tensor("thr", [128, NSUP * 4], f32, kind="ExternalInput")
    npen_d = nc.dram_tensor("npen", [128, NSUP * 4], f32, kind="ExternalInput")
    iota_d = nc.dram_tensor("iota", [128, 256], f32, kind="ExternalInput")
    w1a_d = nc.dram_tensor("w1a", [128, 128], bf16, kind="ExternalInput")
    w1b_d = nc.dram_tensor("w1b", [128, 128], bf16, kind="ExternalInput")
    a1a_d = nc.dram_tensor("a1a", [128, 128], bf16, kind="ExternalInput")
    a1b_d = nc.dram_tensor("a1b", [128, 128], bf16, kind="ExternalInput")
    m2a_d = nc.dram_tensor("m2a", [128, 65], f32, kind="ExternalInput")
    m2b_d = nc.dram_tensor("m2b", [128, 65], f32, kind="ExternalInput")
    b1_d = nc.dram_tensor("b1c", [128, 1], f32, kind="ExternalInput")
    ab1_d = nc.dram_tensor("ab1c", [128, 1], f32, kind="ExternalInput")
    b2c_d = nc.dram_tensor("b2c", [65, 1], f32, kind="ExternalInput")
    out_d = nc.dram_tensor("out65", [EP, 65], f32, kind="ExternalOutput")

    with tile.TileContext(nc) as tc:
        with (
            tc.tile_pool(name="const", bufs=1) as cp,
            tc.tile_pool(name="gat", bufs=2) as gp,
            tc.tile_pool(name="work", bufs=2) as wp,
            tc.tile_pool(name="ps", bufs=1, space="PSUM") as pp,
            tc.tile_pool(name="ps2", bufs=2, space="PSUM") as pp2,
        ):
            nc.gpsimd.load_library(mlp)
            # ---- constants
            sidx_t = cp.tile([128, NBAT * IDXC], i16)
            nc.sync.dma_start(sidx_t[:], sidx_d[:])
            tidx_t = cp.tile([128, NBAT * IDXC], i16)
            nc.sync.dma_start(tidx_t[:], tidx_d[:])
            thr_t = cp.tile([128, NSUP * 4], f32)
            nc.sync.dma_start(thr_t[:], thr_d[:])
            npen_t = cp.tile([128, NSUP * 4], f32)
            nc.sync.dma_start(npen_t[:], npen_d[:])
            iota_t = cp.tile([128, 256], f32)
            nc.sync.dma_start(iota_t[:], iota_d[:])
            w1a_t = cp.tile([128, 128], bf16)
            nc.sync.dma_start(w1a_t[:], w1a_d[:])
            w1b_t = cp.tile([128, 128], bf16)
            nc.sync.dma_start(w1b_t[:], w1b_d[:])
            a1a_t = cp.tile([128, 128], bf16)
            nc.sync.dma_start(a1a_t[:], a1a_d[:])
            a1b_t = cp.tile([128, 128], bf16)
            nc.sync.dma_start(a1b_t[:], a1b_d[:])
            m2a_f = cp.tile([128, 65], f32)
            nc.sync.dma_start(m2a_f[:], m2a_d[:])
            m2b_f = cp.tile([128, 65], f32)
            nc.sync.dma_start(m2b_f[:], m2b_d[:])
            m2a_t = cp.tile([128, 65], f32r)
            nc.vector.tensor_copy(m2a_t[:], m2a_f[:])
            m2b_t = cp.tile([128, 65], f32r)
            nc.vector.tensor_copy(m2b_t[:], m2b_f[:])
            b1_t = cp.tile([128, 1], f32)
            nc.sync.dma_start(b1_t[:], b1_d[:])
            ab1_t = cp.tile([128, 1], f32)
            nc.sync.dma_start(ab1_t[:], ab1_d[:])
            b2c_t = cp.tile([65, 1], f32)
            nc.sync.dma_start(b2c_t[:], b2c_d[:])
            ident_t = cp.tile([128, 128], bf16)
            make_identity(nc, ident_t[:])
            id65_t = cp.tile([65, 65], f32)
            make_identity(nc, id65_t[:])

            for b in range(int(os.environ.get('KERNEL_NBAT', NBAT))):
                seg = b // BPS
                sh, th = seg >> 1, seg & 1
                gsrc = gp.tile([128, BATCH // 128, 128], bf16, tag="gsrc")
                nc.gpsimd.dma_gather(
                    gsrc[:], node_d[SBASE[sh]:, :],
                    sidx_t[:, b * IDXC:(b + 1) * IDXC], BATCH, BATCH, 128,
                    single_packet=False,
                )
                gtgt = gp.tile([128, BATCH // 128, 128], bf16, tag="gtgt")
                nc.gpsimd.dma_gather(
                    gtgt[:], node_d[SBASE[th]:, :],
                    tidx_t[:, b * IDXC:(b + 1) * IDXC], BATCH, BATCH, 128,
                    single_packet=False,
                )
                for s in range(SPB):
                    st = b * SPB + s
                    # ---- feature-major transposes of gathered rows
                    sT_ps = pp.tile([128, 512], bf16, space="PSUM", tag="sT")
                    tT_ps = pp.tile([128, 512], bf16, space="PSUM", tag="tT")
                    for j in range(4):
                        nc.tensor.transpose(
                            sT_ps[:, 128 * j:128 * (j + 1)],
                            gsrc[:, 4 * s + j, :], ident_t[:])
                    for j in range(4):
                        nc.tensor.transpose(
                            tT_ps[:, 128 * j:128 * (j + 1)],
                            gtgt[:, 4 * s + j, :], ident_t[:])
                    sT = wp.tile([128, 512], bf16, tag="sT_sb")
                    nc.scalar.activation(
                        sT[:], sT_ps[:], mybir.ActivationFunctionType.Copy)
                    tT = wp.tile([128, 512], bf16, tag="tT_sb")
                    nc.scalar.activation(
                        tT[:], tT_ps[:], mybir.ActivationFunctionType.Copy)
                    # ---- layer 1 (both MLPs), hidden-major
                    h1_ps = pp.tile([128, 512], f32, space="PSUM", tag="h1")
                    nc.tensor.matmul(h1_ps[:], w1a_t[:], sT[:], start=True, stop=False)
                    nc.tensor.matmul(h1_ps[:], w1b_t[:], tT[:], start=False, stop=True)
                    h2_ps = pp.tile([128, 512], f32, space="PSUM", tag="h2")
                    nc.tensor.matmul(h2_ps[:], a1a_t[:], sT[:], start=True, stop=False)
                    nc.tensor.matmul(h2_ps[:], a1b_t[:], tT[:], start=False, stop=True)
                    h1 = wp.tile([128, 512], f32r, tag="h1_sb")
                    nc.scalar.activation(
                        h1[:], h1_ps[:], mybir.ActivationFunctionType.Relu,
                        bias=b1_t[:, 0:1])
                    h2 = wp.tile([128, 512], f32r, tag="h2_sb")
                    nc.vector.tensor_scalar(
                        h2[:], h2_ps[:], ab1_t[:, 0:1], 0.0,
                        mybir.AluOpType.add, mybir.AluOpType.max)
                    # ---- layer 2 fused: rows 0..63 army, row 64 edge logit
                    o_ps = pp.tile([65, 512], f32, space="PSUM", tag="o65")
                    nc.tensor.matmul(o_ps[:], m2a_t[:], h1[:], start=True, stop=False)
                    nc.tensor.matmul(o_ps[:], m2b_t[:], h2[:], start=False, stop=True)
                    o_sb = wp.tile([65, 512], f32, tag="o_sb")
                    nc.scalar.activation(
                        o_sb[:], o_ps[:], mybir.ActivationFunctionType.Identity,
                        bias=b2c_t[:, 0:1])
                    # ---- back to edge-major, paired so DMA runs are 520B
                    fin_ps = pp2.tile([128, 260], f32, space="PSUM", tag="fin")
                    for g in range(2):
                        for c in range(2):
                            q = 2 * g + c
                            nc.tensor.transpose(
                                fin_ps[:, 65 * q:65 * q + 65],
                                o_sb[:, 256 * g + c:256 * (g + 1):2],
                                id65_t[:])
                    # ---- additive mask: addm[:, 65q+j] = -1e9*(j > thr_q), el col = npen
                    addm = wp.tile([128, 260], f32, tag="addm")
                    for q in range(4):
                        nc.vector.tensor_scalar(
                            addm[:, 65 * q:65 * q + 64],
                            iota_t[:, 0:64],
                            thr_t[:, 4 * st + q:4 * st + q + 1], -1.0e9,
                            mybir.AluOpType.is_gt, mybir.AluOpType.mult)
                    nc.vector.tensor_copy(
                        addm[:].rearrange("p (q x) -> p q x", q=4)[:, :, 64:65],
                        npen_t[:, 4 * st:4 * st + 4].broadcast_to([128, 4, 1]))
                    fin = wp.tile([128, 260], f32, tag="fin_sb")
                    nc.vector.tensor_tensor(
                        fin[:], fin_ps[:], addm[:], mybir.AluOpType.add)
                    # ---- store (rows: 256*g + 2*p + c of this supertile)
                    o_ap = out_d[st * SUP:(st + 1) * SUP, :].rearrange(
                        "(g p c) k -> p g (c k)", g=2, p=128, c=2)
                    nc.sync.dma_start(
                        o_ap, fin[:].rearrange("p (g x) -> p g x", g=2))

    nc.compile()
    _NC_CACHE["nc"] = nc
    return nc


def _prep_core(edges, army_counts):
    """Bucket/pad one core's edges; build all per-core device inputs."""
    src = edges[:, 0].astype(np.int64)
    tgt = edges[:, 1].astype(np.int64)
    sh = (src >= SPLIT).astype(np.int64)
    th = (tgt >= SPLIT).astype(np.int64)
    seg = 2 * sh + th
    order = np.argsort(seg, kind="stable")  # bucketed position -> original idx
    counts = np.bincount(seg, minlength=4)
    if counts.max() > SEG_CAP:
        raise RuntimeError(f"bucket overflow: {counts}")

    src_p = np.empty(EP, np.int64)
    tgt_p = np.empty(EP, np.int64)
    base_s = np.empty(EP, np.int64)
    base_t = np.empty(EP, np.int64)
    thr_p = np.zeros(EP, np.float32)
    npen_p = np.zeros(EP, np.float32)
    slot_of_edge = np.empty(EC, np.int64)

    sa = army_counts[src].astype(np.float32)
    ta = army_counts[tgt].astype(np.float32)
    bad = ((sa <= 2) | (ta >= 3 * sa)).astype(np.float32)
    selfe = (src == tgt).astype(np.float32)
    thr_e = sa - 1.0
    npen_e = -(bad + 100.0 * selfe)

    ofs = 0
    for q in range(4):
        qs, qt = q >> 1, q & 1
        lo = q * SEG_CAP
        n = counts[q]
        idxq = order[ofs:ofs + n]
        ofs += n
        sl = slice(lo, lo + n)
        src_p[sl] = src[idxq]
        tgt_p[sl] = tgt[idxq]
        thr_p[sl] = thr_e[idxq]
        npen_p[sl] = npen_e[idxq]
        slot_of_edge[idxq] = lo + np.arange(n)
        pad = slice(lo + n, lo + SEG_CAP)
        src_p[pad] = SBASE[qs]
        tgt_p[pad] = SBASE[qt]
        base_s[lo:lo + SEG_CAP] = SBASE[qs]
        base_t[lo:lo + SEG_CAP] = SBASE[qt]

    si = src_p - base_s  # int16-range signed offsets
    ti = tgt_p - base_t
    # ensure the LAST slot of every gather batch has non-negative src AND tgt
    # offsets (trailing-negative indices truncate the gather on device)
    inv_slot = None
    for b in range(NBAT):
        last = (b + 1) * BATCH - 1
        if si[last] >= 0 and ti[last] >= 0:
            continue
        blk = slice(b * BATCH, (b + 1) * BATCH)
        cand = np.nonzero((si[blk] >= 0) & (ti[blk] >= 0))[0]
        if cand.size == 0:
            raise RuntimeError("no swap candidate in batch")
        j = b * BATCH + cand[0]
        for arr in (src_p, tgt_p, si, ti, thr_p, npen_p):
            arr[j], arr[last] = arr[last], arr[j]
        if inv_slot is None:
            inv_slot = np.full(EP, -1, np.int64)
            inv_slot[slot_of_edge] = np.arange(EC)
        ej, el = inv_slot[j], inv_slot[last]
        if ej >= 0:
            slot_of_edge[ej] = last
        if el >= 0:
            slot_of_edge[el] = j
        inv_slot[j], inv_slot[last] = el, ej

    def wrap_idx(a):
        cols = a.astype(np.int16).reshape(-1, 16).T  # [16, EP/16]
        return np.ascontiguousarray(np.tile(cols, (8, 1)))  # [128, EP/16]

    # [p, 4*st + 2*g + c] <- edge 512*st + 256*g + 2*p + c
    def edge_tile(a):
        t = a.reshape(NSUP, 2, 128, 2)
        return np.ascontiguousarray(t.transpose(2, 0, 1, 3).reshape(128, NSUP * 4))

    return {
        "sidx": wrap_idx(si),
        "tidx": wrap_idx(ti),
        "thr": edge_tile(thr_p),
        "npen": edge_tile(npen_p),
    }, slot_of_edge


def kernel(node_embeddings, action_edges, army_counts,
           W1, b1, W2, b2, A1, ab1, A2, ab2):
    node_bf = np.asarray(node_embeddings, np.float32).astype(BF16)
    W1 = np.asarray(W1, np.float32)
    A1 = np.asarray(A1, np.float32)
    m2a = np.zeros((128, 65), np.float32)
    m2a[:, 64] = np.asarray(W2, np.float32)[:, 0]
    m2b = np.zeros((128, 65), np.float32)
    m2b[:, :64] = np.asarray(A2, np.float32)
    b2c = np.concatenate(
        [np.asarray(ab2, np.float32), np.asarray(b2, np.float32)])[:, None]
    iota = np.tile(np.arange(64, dtype=np.float32), 4)[None, :].repeat(128, 0)
    shared = {
        "node": node_bf,
        "iota": np.ascontiguousarray(iota),
        "w1a": W1[:128].astype(BF16), "w1b": W1[128:].astype(BF16),
        "a1a": A1[:128].astype(BF16), "a1b": A1[128:].astype(BF16),
        "m2a": m2a, "m2b": m2b,
        "b1c": np.asarray(b1, np.float32)[:, None],
        "ab1c": np.asarray(ab1, np.float32)[:, None],
        "b2c": b2c,
    }

    in_maps = []
    slots = []
    ac = np.asarray(army_counts)
    edges = np.asarray(action_edges)
    for c in range(NCORES):
        per, slot_of_edge = _prep_core(edges[c * EC:(c + 1) * EC], ac)
        in_maps.append({**shared, **per})
        slots.append(slot_of_edge)

    global _LAST_IN_MAPS
    _LAST_IN_MAPS = in_maps
    nc = _build_nc()
    res = run_bass_kernel_spmd(nc, in_maps, core_ids=list(range(NCORES)))

    edge_logits = np.empty(E, np.float32)
    army_logits = np.empty((E, K), np.float32)
    for c in range(NCORES):
        out65 = res.results[c]["out65"]
        sl = slots[c]
        edge_logits[c * EC:(c + 1) * EC] = out65[sl, 64]
        army_logits[c * EC:(c + 1) * EC] = out65[sl, :64]
    return edge_logits, army_logits
